# revision 20
# baseline (speedup 1.0000x reference)
"""Trainium2 Bass kernel for nn_CrispToFuzzyConv (hypergraph message passing).

v3: segment sums as one-hot matmuls on the PE (no DMA scatter-adds),
bf16 dataflow, and block-PAIR gather calls (K=512) to halve the SWDGE
call count -- descriptor generation is effectively serial on the Pool
engine at ~1us fixed cost per call, so call count is the lever.

  Stage A (edges sharded, 2 regions x 25 blocks of 128 edges/core,
    processed as 13 pairs/region): per (pair, X-chunk of 4) one
    dma_gather of K=512 token slots of X[vertex] (tokens sorted by
    edge; -1 padding and exact per-call counts equalized across the 8
    SPMD cores). Per (subtile, spanned-block) a DVE is_equal builds a
    one-hot [128,128] and one PE matmul accumulates into that block's
    PSUM tile: Xe_block = sum onehot^T @ dat. Stored to xe_sum,
    AllGather per region -> xe_tbl[r] [25600,128] bf16.
  Stage C (nodes sharded, 98 blocks as 49 pairs): two passes so the
    region-1 AllGather hides behind region-0 work. Per (pair, region)
    one gather of K=512 from xe_tbl[r]; matmuls dat^T @ onehot ->
    Xv2^T per node block (pass 1 partial parked in SBUF). Pass 2 adds
    region 1 in PSUM, then the dense head per block:
      h2T = psum + partial (Xv2^T), h1T = transpose(deg * X tile),
      a*T = |.|, 6 bf16 matmuls with [256,128] weights split in two,
      biases folded (bias_l = b_b - b_a with w_a negated), out3 write.

Known hardware constraints baked in:
  - gather indices are int16 -> X gathered in 4 chunks of 25000 rows;
    xe_tbl capped at 25600 rows; <= 1024 indices per call
  - gather layout: token t -> partition t%128, column-block t//128
  - num_idxs_reg must equal the exact non-negative index count (the
    ucode generates exactly that many descriptors; mismatch hangs) and
    is baked per instruction, hence the cross-core equalization pads
  - pad slots are never written by the gather: dat pool buffers are
    memset once so 0*stale cannot be NaN
  - collective in/out tensors must be Internal, addr_space Local
"""

import numpy as np

# ---------------------------------------------------------------- constants
N = 100000
E = 50000
NNZ = 300000
F = 128
NC = 8

EDGE_SH = E // NC            # 6250
NODE_SH = N // NC            # 12500
REG = EDGE_SH // 2           # 3125 edges per region
BLK_A = 25                   # 128-edge blocks per (core, region)
PAIR_A = (BLK_A + 1) // 2    # 13 pairs per region (last is a single)
ROWS_REG = BLK_A * 128       # 3200 padded rows per (core, region)
XE_TBL = NC * ROWS_REG       # 25600 rows per region table (int16-safe)
CH = 4                       # X chunks (int16 gather limit)
CHROWS = N // CH             # 25000
KA = 512                     # slots per stage-A (pair, chunk) gather
KC = 512                     # slots per stage-C (pair, region) gather
BLK_C = 98                   # 128-node blocks per core
PAIR_C = BLK_C // 2          # 49
NODE_SH_P = BLK_C * 128      # 12544
NG_A = 2 * PAIR_A * CH       # 104 stage-A gather calls per core
NG_C = PAIR_C * 2            # 98 stage-C gather calls per core

_STATE = {}


# ---------------------------------------------------------------- host side
def _wrap16(idx):
    """[n, K] int -> [n, 128, K//16] int16 (idx i at partition i%16, col
    i//16; replicated across the 8 groups of 16 partitions)."""
    n, K = idx.shape
    t = idx.reshape(n, K // 16, 16).transpose(0, 2, 1).astype(np.int16)
    return np.ascontiguousarray(np.tile(t, (1, 8, 1)))


def _pack_calls(per_core, K, n_calls):
    """per_core[m] = list over calls of (gidx array, local rowid array,
    block-within-pair array (0/1)), tokens block-sorted within the call.
    Returns (equalized counts, wrapped idx per core, entry plan
    [(call, subtile, j, col)], rowids per core [128, n_entries]),
    or None on capacity overflow."""
    cnt = np.zeros(n_calls, np.int64)
    for m in range(NC):
        for g, (gi, _, _) in enumerate(per_core[m]):
            if len(gi) > K:
                return None
            cnt[g] = max(cnt[g], len(gi))
    cnt = np.maximum(cnt, 16)
    # union entry plan across cores: (call g, subtile k, block j) is an
    # entry if any core has a token of block j in subtile k of call g
    present = np.zeros((n_calls, K // 128, 2), bool)
    for m in range(NC):
        for g, (gi, _, bj) in enumerate(per_core[m]):
            slot = np.arange(len(gi))
            present[g, slot // 128, bj] = True
    entries = []
    colmap = {}
    for g in range(n_calls):
        for k in range(K // 128):
            for j in range(2):
                if present[g, k, j]:
                    colmap[(g, k, j)] = len(entries)
                    entries.append((g, k, j, len(entries)))
    ne = len(entries)
    ia_all, row_all = [], []
    for m in range(NC):
        ia = np.full((n_calls, K), -1, np.int64)
        rows = np.full((ne, 128), -1.0, np.float32)
        for g, (gi, ri, bj) in enumerate(per_core[m]):
            n = len(gi)
            ia[g, :n] = gi
            ia[g, n:cnt[g]] = 0
            for t in range(n):
                col = colmap[(g, t // 128, bj[t])]
                rows[col, t % 128] = ri[t]
        ia_all.append(np.ascontiguousarray(
            _wrap16(ia).transpose(1, 0, 2).reshape(128, n_calls * (K // 16))))
        row_all.append(np.ascontiguousarray(rows.T))
    return cnt, ia_all, entries, row_all


def _route(vertex, edges):
    le = edges % EDGE_SH
    owner_a = edges // EDGE_SH
    reg = le // REG
    loc_r = le - reg * REG
    blk_a = loc_r // 128
    row_a = (loc_r - blk_a * 128).astype(np.float32)
    chunk = vertex // CHROWS
    gidx_a = vertex - chunk * CHROWS
    owner_c = vertex // NODE_SH
    loc_c = vertex - owner_c * NODE_SH
    blk_c = loc_c // 128
    row_c = (loc_c - blk_c * 128).astype(np.float32)
    gidx_c = owner_a * ROWS_REG + loc_r

    per_core_a, per_core_c = [], []
    for m in range(NC):
        calls = []
        sel0 = np.nonzero(owner_a == m)[0]
        # sort tokens by (call key, edge) so blocks are contiguous
        key = ((reg[sel0] * PAIR_A + blk_a[sel0] // 2) * CH
               + chunk[sel0]) * (10 ** 6) + le[sel0]
        sel = sel0[np.argsort(key, kind="stable")]
        ck = (reg[sel] * PAIR_A + blk_a[sel] // 2) * CH + chunk[sel]
        starts = np.searchsorted(ck, np.arange(NG_A + 1))
        for g in range(NG_A):
            s = sel[starts[g]:starts[g + 1]]
            calls.append((gidx_a[s], row_a[s], (blk_a[s] % 2).astype(int)))
        per_core_a.append(calls)

        calls = []
        sel0 = np.nonzero(owner_c == m)[0]
        key = ((blk_c[sel0] // 2) * 2 + reg[sel0]) * (10 ** 6) + loc_c[sel0]
        sel = sel0[np.argsort(key, kind="stable")]
        ck = (blk_c[sel] // 2) * 2 + reg[sel]
        starts = np.searchsorted(ck, np.arange(NG_C + 1))
        for g in range(NG_C):
            s = sel[starts[g]:starts[g + 1]]
            calls.append((gidx_c[s], row_c[s], (blk_c[s] % 2).astype(int)))
        per_core_c.append(calls)

    pa = _pack_calls(per_core_a, KA, NG_A)
    pc = _pack_calls(per_core_c, KC, NG_C)
    if pa is None or pc is None:
        return None
    return pa, pc


def _numpy_fallback(X, vertex, edges, w_b, w_a, w_c, b_b, b_a, b_c):
    Xe = np.zeros((E, F), np.float32)
    np.add.at(Xe, edges, X[vertex])
    Xv2 = np.zeros((N, F), np.float32)
    np.add.at(Xv2, vertex, Xe[edges])
    deg = np.bincount(vertex, minlength=N).astype(np.float32)[:, None]
    Xv = np.concatenate([deg * X, Xv2], axis=1)
    center = Xv @ w_b + b_b
    aXv = np.abs(Xv)
    return (center.astype(np.float32),
            (center - (aXv @ w_a + b_a)).astype(np.float32),
            (center + (aXv @ w_c + b_c)).astype(np.float32))


# ------------------------------------------------------------- bass program
def _build_program(cnt_a, ent_a, ne_a, cnt_c, ent_c, ne_c):
    from concourse import bacc, tile
    import concourse.mybir as mybir

    f32 = mybir.dt.float32
    bf16 = mybir.dt.bfloat16
    i16 = mybir.dt.int16

    nc = bacc.Bacc(None, target_bir_lowering=False, debug=False,
                   num_devices=NC, num_swdge_queues=4)

    xfull = nc.dram_tensor("xfull", [N, F], bf16, kind="ExternalInput")
    xshard = nc.dram_tensor("xshard", [NODE_SH_P, F], f32, kind="ExternalInput")
    ia = nc.dram_tensor("ia", [128, NG_A * (KA // 16)], i16,
                        kind="ExternalInput")
    ic = nc.dram_tensor("ic", [128, NG_C * (KC // 16)], i16,
                        kind="ExternalInput")
    rowa_d = nc.dram_tensor("rowa", [128, ne_a], f32, kind="ExternalInput")
    rowc_d = nc.dram_tensor("rowc", [128, ne_c], f32, kind="ExternalInput")
    deg = nc.dram_tensor("deg", [128, BLK_C], f32, kind="ExternalInput")
    wts_d = {nm: nc.dram_tensor(nm, [F, F], bf16, kind="ExternalInput")
             for nm in ("wb1", "wb2", "wa1n", "wa2n", "wc1", "wc2")}
    bias_d = {nm: nc.dram_tensor(nm, [1, F], f32, kind="ExternalInput")
              for nm in ("bias_c", "bias_l", "bias_r")}
    out3 = nc.dram_tensor("out3", [NODE_SH_P, 3 * F], f32,
                          kind="ExternalOutput")

    xe_sum = nc.dram_tensor("xe_sum", [2 * ROWS_REG, F], bf16)
    xe_tbl = [nc.dram_tensor(f"xe_tbl{r}", [XE_TBL, F], bf16)
              for r in range(2)]

    eye_d = nc.inline_tensor(np.eye(128, dtype=np.float32), name="eye128")
    ramp_d = nc.inline_tensor(
        np.broadcast_to(np.arange(128, dtype=np.float32), (128, 128)).copy(),
        name="ramp")

    ISEQ = mybir.AluOpType.is_equal
    Abs = mybir.ActivationFunctionType.Abs
    Copy = mybir.ActivationFunctionType.Copy

    def group_entries(entries, calls_per_group):
        groups = {}
        for g, k, j, col in entries:
            groups.setdefault(g // calls_per_group, []).append((g, k, j, col))
        return groups

    ga = group_entries(ent_a, CH)        # per (region, pair)
    gc = group_entries(ent_c, 1)         # per (pair, region) call

    with tile.TileContext(nc) as tc:
        with (
            tc.tile_pool(name="cpool", bufs=1) as cpool,
            tc.tile_pool(name="ppool", bufs=1) as ppool,
            tc.tile_pool(name="dpa", bufs=4) as dpa,
            tc.tile_pool(name="oha", bufs=8) as oha,
            tc.tile_pool(name="dpc", bufs=6) as dpc,
            tc.tile_pool(name="ohc", bufs=8) as ohc,
            tc.tile_pool(name="spool", bufs=6) as spool,
            tc.tile_pool(name="opool", bufs=3) as opool,
            tc.tile_pool(name="ps_sg", bufs=4, space="PSUM") as ps_sg,
            tc.tile_pool(name="ps_tr", bufs=1, space="PSUM") as ps_tr,
            tc.tile_pool(name="ps_mm", bufs=3, space="PSUM") as ps_mm,
        ):
            ident = cpool.tile([128, 128], f32)
            nc.sync.dma_start(ident[:], eye_d[:])
            ramp = cpool.tile([128, 128], f32)
            nc.sync.dma_start(ramp[:], ramp_d[:])
            rowa_s = cpool.tile([128, ne_a], f32)
            nc.sync.dma_start(rowa_s[:], rowa_d[:])
            rowc_s = cpool.tile([128, ne_c], f32)
            nc.sync.dma_start(rowc_s[:], rowc_d[:])
            deg_all = cpool.tile([128, BLK_C], f32)
            nc.sync.dma_start(deg_all[:], deg[:])
            iat = cpool.tile([128, NG_A * (KA // 16)], i16)
            nc.sync.dma_start(iat[:], ia[:])
            ict = cpool.tile([128, NG_C * (KC // 16)], i16)
            nc.sync.dma_start(ict[:], ic[:])
            ones = cpool.tile([1, F], f32)
            nc.vector.memset(ones[:], 1.0)
            wts = {}
            for nm, d in wts_d.items():
                wtile = cpool.tile([F, F], bf16, tag=nm)
                nc.sync.dma_start(wtile[:], d[:])
                wts[nm] = wtile
            bias_bc = {}
            bmm = ps_mm.tile([128, 3 * F], f32, tag="mm3")
            for k, (nm, d) in enumerate(bias_d.items()):
                btile = cpool.tile([1, F], f32, tag=nm)
                nc.sync.dma_start(btile[:], d[:])
                nc.tensor.matmul(bmm[:, k * F:(k + 1) * F], ones[:], btile[:],
                                 start=True, stop=True)
            for k, nm in enumerate(bias_d):
                bct = cpool.tile([128, F], f32, tag=f"bc_{nm}")
                nc.vector.tensor_copy(bct[:], bmm[:, k * F:(k + 1) * F])
                bias_bc[nm] = bct
            for _ in range(4):
                t = dpa.tile([128, 4 * CH, F], bf16, tag="datA")
                nc.vector.memset(t[:], 0.0)
            for _ in range(6):
                t = dpc.tile([128, 4, F], bf16, tag="datC")
                nc.vector.memset(t[:], 0.0)

            def cc(r):
                lo, hi = r * ROWS_REG, (r + 1) * ROWS_REG
                nc.gpsimd.collective_compute(
                    "AllGather", mybir.AluOpType.bypass,
                    replica_groups=[list(range(NC))],
                    ins=[xe_sum[lo:hi, :].opt()],
                    outs=[xe_tbl[r].ap().opt()],
                )

            # stage A
            for r in range(2):
                for p in range(PAIR_A):
                    grp = r * PAIR_A + p
                    nblk = 1 if 2 * p + 1 >= BLK_A else 2
                    dat = dpa.tile([128, 4 * CH, F], bf16, tag="datA")
                    for c in range(CH):
                        g = grp * CH + c
                        nc.gpsimd.dma_gather(
                            dat[:, 4 * c:4 * c + 4, :],
                            xfull[c * CHROWS:(c + 1) * CHROWS, :],
                            iat[:, g * (KA // 16):(g + 1) * (KA // 16)],
                            KA, int(cnt_a[g]), F, queue_num=g % 4)
                    ents = ga.get(grp, [])
                    ps = [ps_sg.tile([128, F], f32, tag="sg",
                                     name=f"psA{r}_{p}_{jj}")
                          for jj in range(nblk)]
                    nj = [sum(1 for e in ents if e[2] == j)
                          for j in range(nblk)]
                    seen = [0, 0]
                    for g, k, j, col in ents:
                        oh = oha.tile([128, 128], bf16, tag="ohA")
                        nc.vector.tensor_scalar(
                            oh[:], ramp[:], rowa_s[:, col:col + 1], None,
                            ISEQ)
                        c = g % CH
                        nc.tensor.matmul(ps[j][:], oh[:],
                                         dat[:, 4 * c + k, :],
                                         start=(seen[j] == 0),
                                         stop=(seen[j] == nj[j] - 1))
                        seen[j] += 1
                    for j in range(nblk):
                        st = spool.tile([128, F], bf16, tag="xe_st")
                        nc.scalar.activation(st[:], ps[j][:], Copy)
                        row0 = r * ROWS_REG + (2 * p + j) * 128
                        nc.scalar.dma_start(xe_sum[row0:row0 + 128, :], st[:])
                    if r == 1 and p == 1:
                        cc(0)
            cc(1)

            # stage C pass 1 (region 0) -> SBUF partials
            parts = {}
            for p in range(PAIR_C):
                g = p * 2
                dat = dpc.tile([128, 4, F], bf16, tag="datC")
                nc.gpsimd.dma_gather(
                    dat[:], xe_tbl[0][:],
                    ict[:, g * (KC // 16):(g + 1) * (KC // 16)],
                    KC, int(cnt_c[g]), F, queue_num=p % 4)
                ents = gc.get(g, [])
                ps = [ps_sg.tile([128, 128], f32, tag="sg",
                                 name=f"psC1_{p}_{jj}") for jj in range(2)]
                nj = [sum(1 for e in ents if e[2] == j) for j in range(2)]
                seen = [0, 0]
                for _, k, j, col in ents:
                    oh = ohc.tile([128, 128], bf16, tag="ohC")
                    nc.vector.tensor_scalar(
                        oh[:], ramp[:], rowc_s[:, col:col + 1], None, ISEQ)
                    nc.tensor.matmul(ps[j][:], dat[:, k, :], oh[:],
                                     start=(seen[j] == 0),
                                     stop=(seen[j] == nj[j] - 1))
                    seen[j] += 1
                for j in range(2):
                    part = ppool.tile([128, 128], f32, tag=f"part{2 * p + j}")
                    nc.vector.tensor_copy(part[:], ps[j][:])
                    parts[2 * p + j] = part

            # stage C pass 2 (region 1) + dense head
            for p in range(PAIR_C):
                g = p * 2 + 1
                dat = dpc.tile([128, 4, F], bf16, tag="datC")
                nc.gpsimd.dma_gather(
                    dat[:], xe_tbl[1][:],
                    ict[:, g * (KC // 16):(g + 1) * (KC // 16)],
                    KC, int(cnt_c[g]), F, queue_num=(p + 2) % 4)
                ents = gc.get(g, [])
                ps = [ps_sg.tile([128, 128], f32, tag="sg",
                                 name=f"psC2_{p}_{jj}") for jj in range(2)]
                nj = [sum(1 for e in ents if e[2] == j) for j in range(2)]
                seen = [0, 0]
                for _, k, j, col in ents:
                    oh = ohc.tile([128, 128], bf16, tag="ohC")
                    nc.vector.tensor_scalar(
                        oh[:], ramp[:], rowc_s[:, col:col + 1], None, ISEQ)
                    nc.tensor.matmul(ps[j][:], dat[:, k, :], oh[:],
                                     start=(seen[j] == 0),
                                     stop=(seen[j] == nj[j] - 1))
                    seen[j] += 1
                for j in range(2):
                    b = 2 * p + j
                    h2T = spool.tile([128, 128], bf16, tag="h2T")
                    nc.vector.tensor_add(h2T[:], ps[j][:], parts[b][:])
                    a2T = spool.tile([128, 128], bf16, tag="a2T")
                    nc.scalar.activation(a2T[:], h2T[:], Abs)

                    r0 = b * 128
                    xt = spool.tile([128, F], f32, tag="xt")
                    nc.sync.dma_start(xt[:], xshard[r0:r0 + 128, :])
                    h1 = spool.tile([128, F], f32, tag="h1")
                    nc.vector.tensor_scalar_mul(h1[:], xt[:],
                                                deg_all[:, b:b + 1])
                    h1T_ps = ps_tr.tile([128, F], f32, tag="h1T_ps")
                    nc.tensor.transpose(h1T_ps[:], h1[:], ident[:])
                    h1T = spool.tile([128, F], bf16, tag="h1T")
                    nc.vector.tensor_copy(h1T[:], h1T_ps[:])
                    a1T = spool.tile([128, F], bf16, tag="a1T")
                    nc.scalar.activation(a1T[:], h1T_ps[:], Abs)

                    groups = (
                        ("hsum_ps", (("h1T", "wb1"), ("h2T", "wb2"))),
                        ("lpart_ps", (("a1T", "wa1n"), ("a2T", "wa2n"))),
                        ("rpart_ps", (("a1T", "wc1"), ("a2T", "wc2"))),
                    )
                    lhs = {"h1T": h1T, "h2T": h2T, "a1T": a1T, "a2T": a2T}
                    mm = ps_mm.tile([128, 3 * F], f32, tag="mm3")
                    ps_out = {}
                    for kk, (psname, terms) in enumerate(groups):
                        pso = mm[:, kk * F:(kk + 1) * F]
                        for i, (ln, wn) in enumerate(terms):
                            nc.tensor.matmul(
                                pso, lhs[ln][:], wts[wn][:],
                                start=(i == 0), stop=(i == len(terms) - 1))
                        ps_out[psname] = pso
                    ot = opool.tile([128, 3 * F], f32, tag="ot")
                    nc.vector.tensor_add(ot[:, 0:F], bias_bc["bias_c"][:],
                                         ps_out["hsum_ps"])
                    for kk, (pname, bname) in enumerate(
                            (("lpart_ps", "bias_l"), ("rpart_ps", "bias_r"))):
                        tmp = opool.tile([128, F], f32, tag=f"t{kk}")
                        nc.vector.tensor_add(tmp[:], bias_bc[bname][:],
                                             ps_out[pname])
                        nc.vector.tensor_add(
                            ot[:, (kk + 1) * F:(kk + 2) * F],
                            tmp[:], ps_out["hsum_ps"])
                    nc.sync.dma_start(out3[r0:r0 + 128, :], ot[:])

    nc.compile()
    return nc


# ------------------------------------------------------------------- driver
def kernel(X, vertex, edges, X0, n_edges, w_b, w_a, w_c, b_b, b_a, b_c):
    from concourse.bass_utils import run_bass_kernel_spmd
    import ml_dtypes

    X = np.ascontiguousarray(np.asarray(X, dtype=np.float32))
    vertex = np.asarray(vertex).astype(np.int64)
    edges = np.asarray(edges).astype(np.int64)
    w_b = np.asarray(w_b, dtype=np.float32)
    w_a = np.asarray(w_a, dtype=np.float32)
    w_c = np.asarray(w_c, dtype=np.float32)
    b_b = np.asarray(b_b, dtype=np.float32).reshape(1, F)
    b_a = np.asarray(b_a, dtype=np.float32).reshape(1, F)
    b_c = np.asarray(b_c, dtype=np.float32).reshape(1, F)

    r = _route(vertex, edges)
    if r is None:
        return _numpy_fallback(X, vertex, edges, w_b, w_a, w_c, b_b, b_a, b_c)
    (cnt_a, ia_all, ent_a, rowa_all), (cnt_c, ic_all, ent_c, rowc_all) = r

    key = (cnt_a.tobytes(), cnt_c.tobytes(), tuple(ent_a), tuple(ent_c))
    if _STATE.get("key") != key:
        _STATE["nc"] = _build_program(cnt_a, ent_a, len(ent_a),
                                      cnt_c, ent_c, len(ent_c))
        _STATE["key"] = key
    nc = _STATE["nc"]

    Xb = np.ascontiguousarray(X.astype(ml_dtypes.bfloat16))
    deg_full = np.bincount(vertex, minlength=N).astype(np.float32)
    wmats = {
        "wb1": w_b[:F], "wb2": w_b[F:],
        "wa1n": -w_a[:F], "wa2n": -w_a[F:],
        "wc1": w_c[:F], "wc2": w_c[F:],
    }
    bmats = {"bias_c": b_b, "bias_l": b_b - b_a, "bias_r": b_b + b_c}

    in_maps = []
    for m in range(NC):
        xs = np.zeros((NODE_SH_P, F), np.float32)
        xs[:NODE_SH] = X[m * NODE_SH:(m + 1) * NODE_SH]
        dshard = np.zeros(NODE_SH_P, np.float32)
        dshard[:NODE_SH] = deg_full[m * NODE_SH:(m + 1) * NODE_SH]
        im = {
            "xfull": Xb,
            "xshard": xs,
            "ia": ia_all[m], "ic": ic_all[m],
            "rowa": rowa_all[m], "rowc": rowc_all[m],
            "deg": np.ascontiguousarray(dshard.reshape(BLK_C, 128).T),
        }
        for nm, w in wmats.items():
            im[nm] = np.ascontiguousarray(w.astype(ml_dtypes.bfloat16))
        for nm, bv in bmats.items():
            im[nm] = np.ascontiguousarray(bv.astype(np.float32))
        in_maps.append(im)

    res = run_bass_kernel_spmd(nc, in_maps, list(range(NC)))
    full = np.concatenate([res.results[m]["out3"][:NODE_SH]
                           for m in range(NC)])
    full = full.reshape(N, 3, F)
    return (np.ascontiguousarray(full[:, 0]),
            np.ascontiguousarray(full[:, 1]),
            np.ascontiguousarray(full[:, 2]))


# revision 21
# speedup vs baseline: 1.2567x; 1.2567x over previous
"""Trainium2 Bass kernel for nn_CrispToFuzzyConv (hypergraph message passing).

v3: segment sums as one-hot matmuls on the PE (no DMA scatter-adds),
bf16 dataflow, and block-PAIR gather calls (K=512) to halve the SWDGE
call count -- descriptor generation is effectively serial on the Pool
engine at ~1us fixed cost per call, so call count is the lever.

  Stage A (edges sharded, 2 regions x 25 blocks of 128 edges/core,
    processed as 13 pairs/region): per (pair, X-chunk of 4) one
    dma_gather of K=512 token slots of X[vertex] (tokens sorted by
    edge; -1 padding and exact per-call counts equalized across the 8
    SPMD cores). Per (subtile, spanned-block) a DVE is_equal builds a
    one-hot [128,128] and one PE matmul accumulates into that block's
    PSUM tile: Xe_block = sum onehot^T @ dat. Stored to xe_sum,
    AllGather per region -> xe_tbl[r] [25600,128] bf16.
  Stage C (nodes sharded, 98 blocks as 49 pairs): two passes so the
    region-1 AllGather hides behind region-0 work. Per (pair, region)
    one gather of K=512 from xe_tbl[r]; matmuls dat^T @ onehot ->
    Xv2^T per node block (pass 1 partial parked in SBUF). Pass 2 adds
    region 1 in PSUM, then the dense head per block:
      h2T = psum + partial (Xv2^T), h1T = transpose(deg * X tile),
      a*T = |.|, 6 bf16 matmuls with [256,128] weights split in two,
      biases folded (bias_l = b_b - b_a with w_a negated), out3 write.

Known hardware constraints baked in:
  - gather indices are int16 -> X gathered in 4 chunks of 25000 rows;
    xe_tbl capped at 25600 rows; <= 1024 indices per call
  - gather layout: token t -> partition t%128, column-block t//128
  - num_idxs_reg must equal the exact non-negative index count (the
    ucode generates exactly that many descriptors; mismatch hangs) and
    is baked per instruction, hence the cross-core equalization pads
  - pad slots are never written by the gather: dat pool buffers are
    memset once so 0*stale cannot be NaN
  - collective in/out tensors must be Internal, addr_space Local
"""

import numpy as np

# ---------------------------------------------------------------- constants
N = 100000
E = 50000
NNZ = 300000
F = 128
NC = 8

EDGE_SH = E // NC            # 6250
NODE_SH = N // NC            # 12500
REG = EDGE_SH // 2           # 3125 edges per region
BLK_A = 25                   # 128-edge blocks per (core, region)
PAIR_A = (BLK_A + 1) // 2    # 13 pairs per region (last is a single)
ROWS_REG = BLK_A * 128       # 3200 padded rows per (core, region)
XE_TBL = NC * ROWS_REG       # 25600 rows per region table (int16-safe)
CH = 4                       # X chunks (int16 gather limit)
CHROWS = N // CH             # 25000
KA = 512                     # slots per stage-A (pair, chunk) gather
KC = 512                     # slots per stage-C (pair, region) gather
BLK_C = 98                   # 128-node blocks per core
PAIR_C = BLK_C // 2          # 49
NODE_SH_P = BLK_C * 128      # 12544
NG_A = 2 * PAIR_A * CH       # 104 stage-A gather calls per core
NG_C = PAIR_C * 2            # 98 stage-C gather calls per core

_STATE = {}


# ---------------------------------------------------------------- host side
def _wrap16(idx):
    """[n, K] int -> [n, 128, K//16] int16 (idx i at partition i%16, col
    i//16; replicated across the 8 groups of 16 partitions)."""
    n, K = idx.shape
    t = idx.reshape(n, K // 16, 16).transpose(0, 2, 1).astype(np.int16)
    return np.ascontiguousarray(np.tile(t, (1, 8, 1)))


def _pack_calls(per_core, K, n_calls):
    """per_core[m] = list over calls of (gidx array, local rowid array,
    block-within-pair array (0/1)), tokens block-sorted within the call.
    Returns (equalized counts, wrapped idx per core, entry plan
    [(call, subtile, j, col)], rowids per core [128, n_entries]),
    or None on capacity overflow."""
    cnt = np.zeros(n_calls, np.int64)
    for m in range(NC):
        for g, (gi, _, _) in enumerate(per_core[m]):
            if len(gi) > K:
                return None
            cnt[g] = max(cnt[g], len(gi))
    cnt = np.maximum(cnt, 16)
    # union entry plan across cores: (call g, subtile k, block j) is an
    # entry if any core has a token of block j in subtile k of call g
    present = np.zeros((n_calls, K // 128, 2), bool)
    for m in range(NC):
        for g, (gi, _, bj) in enumerate(per_core[m]):
            slot = np.arange(len(gi))
            present[g, slot // 128, bj] = True
    entries = []
    colmap = {}
    for g in range(n_calls):
        for k in range(K // 128):
            for j in range(2):
                if present[g, k, j]:
                    colmap[(g, k, j)] = len(entries)
                    entries.append((g, k, j, len(entries)))
    ne = len(entries)
    ia_all, row_all = [], []
    for m in range(NC):
        ia = np.full((n_calls, K), -1, np.int64)
        rows = np.full((ne, 128), -1.0, np.float32)
        for g, (gi, ri, bj) in enumerate(per_core[m]):
            n = len(gi)
            ia[g, :n] = gi
            ia[g, n:cnt[g]] = 0
            for t in range(n):
                col = colmap[(g, t // 128, bj[t])]
                rows[col, t % 128] = ri[t]
        ia_all.append(np.ascontiguousarray(
            _wrap16(ia).transpose(1, 0, 2).reshape(128, n_calls * (K // 16))))
        row_all.append(np.ascontiguousarray(rows.T))
    return cnt, ia_all, entries, row_all


def _route(vertex, edges):
    le = edges % EDGE_SH
    owner_a = edges // EDGE_SH
    reg = le // REG
    loc_r = le - reg * REG
    blk_a = loc_r // 128
    row_a = (loc_r - blk_a * 128).astype(np.float32)
    chunk = vertex // CHROWS
    gidx_a = vertex - chunk * CHROWS
    owner_c = vertex // NODE_SH
    loc_c = vertex - owner_c * NODE_SH
    blk_c = loc_c // 128
    row_c = (loc_c - blk_c * 128).astype(np.float32)
    gidx_c = owner_a * ROWS_REG + loc_r

    per_core_a, per_core_c = [], []
    for m in range(NC):
        calls = []
        sel0 = np.nonzero(owner_a == m)[0]
        # sort tokens by (call key, edge) so blocks are contiguous
        key = ((reg[sel0] * PAIR_A + blk_a[sel0] // 2) * CH
               + chunk[sel0]) * (10 ** 6) + le[sel0]
        sel = sel0[np.argsort(key, kind="stable")]
        ck = (reg[sel] * PAIR_A + blk_a[sel] // 2) * CH + chunk[sel]
        starts = np.searchsorted(ck, np.arange(NG_A + 1))
        for g in range(NG_A):
            s = sel[starts[g]:starts[g + 1]]
            calls.append((gidx_a[s], row_a[s], (blk_a[s] % 2).astype(int)))
        per_core_a.append(calls)

        calls = []
        sel0 = np.nonzero(owner_c == m)[0]
        key = ((blk_c[sel0] // 2) * 2 + reg[sel0]) * (10 ** 6) + loc_c[sel0]
        sel = sel0[np.argsort(key, kind="stable")]
        ck = (blk_c[sel] // 2) * 2 + reg[sel]
        starts = np.searchsorted(ck, np.arange(NG_C + 1))
        for g in range(NG_C):
            s = sel[starts[g]:starts[g + 1]]
            calls.append((gidx_c[s], row_c[s], (blk_c[s] % 2).astype(int)))
        per_core_c.append(calls)

    pa = _pack_calls(per_core_a, KA, NG_A)
    pc = _pack_calls(per_core_c, KC, NG_C)
    if pa is None or pc is None:
        return None
    return pa, pc


def _numpy_fallback(X, vertex, edges, w_b, w_a, w_c, b_b, b_a, b_c):
    Xe = np.zeros((E, F), np.float32)
    np.add.at(Xe, edges, X[vertex])
    Xv2 = np.zeros((N, F), np.float32)
    np.add.at(Xv2, vertex, Xe[edges])
    deg = np.bincount(vertex, minlength=N).astype(np.float32)[:, None]
    Xv = np.concatenate([deg * X, Xv2], axis=1)
    center = Xv @ w_b + b_b
    aXv = np.abs(Xv)
    return (center.astype(np.float32),
            (center - (aXv @ w_a + b_a)).astype(np.float32),
            (center + (aXv @ w_c + b_c)).astype(np.float32))


# ------------------------------------------------------------- bass program
def _build_program(cnt_a, ent_a, ne_a, cnt_c, ent_c, ne_c):
    from concourse import bacc, tile
    import concourse.mybir as mybir

    f32 = mybir.dt.float32
    bf16 = mybir.dt.bfloat16
    i16 = mybir.dt.int16

    nc = bacc.Bacc(None, target_bir_lowering=False, debug=False,
                   num_devices=NC, num_swdge_queues=4)

    xfull = nc.dram_tensor("xfull", [N, F], bf16, kind="ExternalInput")
    xshard = nc.dram_tensor("xshard", [NODE_SH_P, F], f32, kind="ExternalInput")
    ia = nc.dram_tensor("ia", [128, NG_A * (KA // 16)], i16,
                        kind="ExternalInput")
    ic = nc.dram_tensor("ic", [128, NG_C * (KC // 16)], i16,
                        kind="ExternalInput")
    rowa_d = nc.dram_tensor("rowa", [128, ne_a], f32, kind="ExternalInput")
    rowc_d = nc.dram_tensor("rowc", [128, ne_c], f32, kind="ExternalInput")
    deg = nc.dram_tensor("deg", [128, BLK_C], f32, kind="ExternalInput")
    wts_d = {nm: nc.dram_tensor(nm, [F, F], bf16, kind="ExternalInput")
             for nm in ("wb1", "wb2", "wa1n", "wa2n", "wc1", "wc2")}
    bias_d = {nm: nc.dram_tensor(nm, [1, F], f32, kind="ExternalInput")
              for nm in ("bias_c", "bias_l", "bias_r")}
    out3 = nc.dram_tensor("out3", [NODE_SH_P, 3 * F], f32,
                          kind="ExternalOutput")

    xe_sum = nc.dram_tensor("xe_sum", [2 * ROWS_REG, F], bf16)
    xe_tbl = [nc.dram_tensor(f"xe_tbl{r}", [XE_TBL, F], bf16)
              for r in range(2)]

    eye_d = nc.inline_tensor(np.eye(128, dtype=np.float32), name="eye128")
    ramp_d = nc.inline_tensor(
        np.broadcast_to(np.arange(128, dtype=np.float32),
                        (128, 8, 128)).copy(), name="ramp8")

    ISEQ = mybir.AluOpType.is_equal
    Abs = mybir.ActivationFunctionType.Abs
    Copy = mybir.ActivationFunctionType.Copy

    def group_entries(entries, calls_per_group):
        groups = {}
        for g, k, j, col in entries:
            groups.setdefault(g // calls_per_group, []).append((g, k, j, col))
        return groups

    ga = group_entries(ent_a, CH)        # per (region, pair)
    gc = group_entries(ent_c, 1)         # per (pair, region) call

    with tile.TileContext(nc) as tc:
        with (
            tc.tile_pool(name="cpool", bufs=1) as cpool,
            tc.tile_pool(name="ppool", bufs=1) as ppool,
            tc.tile_pool(name="dpa", bufs=4) as dpa,
            tc.tile_pool(name="oha", bufs=4) as oha,
            tc.tile_pool(name="dpc", bufs=6) as dpc,
            tc.tile_pool(name="ohc", bufs=4) as ohc,
            tc.tile_pool(name="spool", bufs=6) as spool,
            tc.tile_pool(name="opool", bufs=3) as opool,
            tc.tile_pool(name="ps_sg", bufs=4, space="PSUM") as ps_sg,
            tc.tile_pool(name="ps_tr", bufs=1, space="PSUM") as ps_tr,
            tc.tile_pool(name="ps_mm", bufs=3, space="PSUM") as ps_mm,
        ):
            ident = cpool.tile([128, 128], f32)
            nc.sync.dma_start(ident[:], eye_d[:])
            ramp = cpool.tile([128, 8, 128], f32)
            nc.sync.dma_start(ramp[:], ramp_d[:])
            rowa_s = cpool.tile([128, ne_a], f32)
            nc.sync.dma_start(rowa_s[:], rowa_d[:])
            rowc_s = cpool.tile([128, ne_c], f32)
            nc.sync.dma_start(rowc_s[:], rowc_d[:])
            deg_all = cpool.tile([128, BLK_C], f32)
            nc.sync.dma_start(deg_all[:], deg[:])
            iat = cpool.tile([128, NG_A * (KA // 16)], i16)
            nc.sync.dma_start(iat[:], ia[:])
            ict = cpool.tile([128, NG_C * (KC // 16)], i16)
            nc.sync.dma_start(ict[:], ic[:])
            ones = cpool.tile([1, F], f32)
            nc.vector.memset(ones[:], 1.0)
            wts = {}
            for nm, d in wts_d.items():
                wtile = cpool.tile([F, F], bf16, tag=nm)
                nc.sync.dma_start(wtile[:], d[:])
                wts[nm] = wtile
            bias_bc = {}
            bmm = ps_mm.tile([128, 3 * F], f32, tag="mm3")
            for k, (nm, d) in enumerate(bias_d.items()):
                btile = cpool.tile([1, F], f32, tag=nm)
                nc.sync.dma_start(btile[:], d[:])
                nc.tensor.matmul(bmm[:, k * F:(k + 1) * F], ones[:], btile[:],
                                 start=True, stop=True)
            for k, nm in enumerate(bias_d):
                bct = cpool.tile([128, F], f32, tag=f"bc_{nm}")
                nc.vector.tensor_copy(bct[:], bmm[:, k * F:(k + 1) * F])
                bias_bc[nm] = bct
            for _ in range(4):
                t = dpa.tile([128, 4 * CH, F], bf16, tag="datA")
                nc.vector.memset(t[:], 0.0)
            for _ in range(6):
                t = dpc.tile([128, 4, F], bf16, tag="datC")
                nc.vector.memset(t[:], 0.0)

            def cc(r):
                lo, hi = r * ROWS_REG, (r + 1) * ROWS_REG
                nc.gpsimd.collective_compute(
                    "AllGather", mybir.AluOpType.bypass,
                    replica_groups=[list(range(NC))],
                    ins=[xe_sum[lo:hi, :].opt()],
                    outs=[xe_tbl[r].ap().opt()],
                )

            # stage A
            for r in range(2):
                for p in range(PAIR_A):
                    grp = r * PAIR_A + p
                    nblk = 1 if 2 * p + 1 >= BLK_A else 2
                    dat = dpa.tile([128, 4 * CH, F], bf16, tag="datA")
                    for c in range(CH):
                        g = grp * CH + c
                        nc.gpsimd.dma_gather(
                            dat[:, 4 * c:4 * c + 4, :],
                            xfull[c * CHROWS:(c + 1) * CHROWS, :],
                            iat[:, g * (KA // 16):(g + 1) * (KA // 16)],
                            KA, int(cnt_a[g]), F, queue_num=g % 4)
                    ents = ga.get(grp, [])
                    ps = [ps_sg.tile([128, F], f32, tag="sg",
                                     name=f"psA{r}_{p}_{jj}")
                          for jj in range(nblk)]
                    nj = [sum(1 for e in ents if e[2] == j)
                          for j in range(nblk)]
                    col0 = ents[0][3]
                    nent = len(ents)
                    for e0 in range(0, nent, 8):
                        ne8 = min(8, nent - e0)
                        oh = oha.tile([128, 8, 128], bf16, tag="ohA")
                        nc.vector.tensor_tensor(
                            oh[:, :ne8, :], ramp[:, :ne8, :],
                            rowa_s[:, col0 + e0:col0 + e0 + ne8]
                            .unsqueeze(2).broadcast_to((128, ne8, 128)),
                            ISEQ)
                        seen = [sum(1 for e in ents[:e0] if e[2] == jj)
                                for jj in range(2)]
                        for g, k, j, col in ents[e0:e0 + ne8]:
                            c = g % CH
                            nc.tensor.matmul(ps[j][:],
                                             oh[:, col - col0 - e0, :],
                                             dat[:, 4 * c + k, :],
                                             start=(seen[j] == 0),
                                             stop=(seen[j] == nj[j] - 1))
                            seen[j] += 1
                    for j in range(nblk):
                        st = spool.tile([128, F], bf16, tag="xe_st")
                        nc.scalar.activation(st[:], ps[j][:], Copy)
                        row0 = r * ROWS_REG + (2 * p + j) * 128
                        nc.scalar.dma_start(xe_sum[row0:row0 + 128, :], st[:])
                    if r == 1 and p == 1:
                        cc(0)
            cc(1)

            # stage C pass 1 (region 0) -> SBUF partials
            parts = {}
            for p in range(PAIR_C):
                g = p * 2
                dat = dpc.tile([128, 4, F], bf16, tag="datC")
                nc.gpsimd.dma_gather(
                    dat[:], xe_tbl[0][:],
                    ict[:, g * (KC // 16):(g + 1) * (KC // 16)],
                    KC, int(cnt_c[g]), F, queue_num=p % 4)
                ents = gc.get(g, [])
                ps = [ps_sg.tile([128, 128], f32, tag="sg",
                                 name=f"psC1_{p}_{jj}") for jj in range(2)]
                nj = [sum(1 for e in ents if e[2] == j) for j in range(2)]
                col0 = ents[0][3]
                oh = ohc.tile([128, 8, 128], bf16, tag="ohC")
                nc.vector.tensor_tensor(
                    oh[:, :len(ents), :], ramp[:, :len(ents), :],
                    rowc_s[:, col0:col0 + len(ents)].unsqueeze(2)
                    .broadcast_to((128, len(ents), 128)), ISEQ)
                seen = [0, 0]
                for _, k, j, col in ents:
                    nc.tensor.matmul(ps[j][:], dat[:, k, :],
                                     oh[:, col - col0, :],
                                     start=(seen[j] == 0),
                                     stop=(seen[j] == nj[j] - 1))
                    seen[j] += 1
                for j in range(2):
                    part = ppool.tile([128, 128], f32, tag=f"part{2 * p + j}")
                    nc.scalar.activation(part[:], ps[j][:], Copy)
                    parts[2 * p + j] = part

            # stage C pass 2 (region 1) + dense head
            for p in range(PAIR_C):
                g = p * 2 + 1
                dat = dpc.tile([128, 4, F], bf16, tag="datC")
                nc.gpsimd.dma_gather(
                    dat[:], xe_tbl[1][:],
                    ict[:, g * (KC // 16):(g + 1) * (KC // 16)],
                    KC, int(cnt_c[g]), F, queue_num=(p + 2) % 4)
                ents = gc.get(g, [])
                ps = [ps_sg.tile([128, 128], f32, tag="sg",
                                 name=f"psC2_{p}_{jj}") for jj in range(2)]
                nj = [sum(1 for e in ents if e[2] == j) for j in range(2)]
                col0 = ents[0][3]
                oh = ohc.tile([128, 8, 128], bf16, tag="ohC")
                nc.vector.tensor_tensor(
                    oh[:, :len(ents), :], ramp[:, :len(ents), :],
                    rowc_s[:, col0:col0 + len(ents)].unsqueeze(2)
                    .broadcast_to((128, len(ents), 128)), ISEQ)
                seen = [0, 0]
                for _, k, j, col in ents:
                    nc.tensor.matmul(ps[j][:], dat[:, k, :],
                                     oh[:, col - col0, :],
                                     start=(seen[j] == 0),
                                     stop=(seen[j] == nj[j] - 1))
                    seen[j] += 1
                for j in range(2):
                    b = 2 * p + j
                    h2T = spool.tile([128, 128], bf16, tag="h2T")
                    nc.vector.tensor_add(h2T[:], ps[j][:], parts[b][:])
                    a2T = spool.tile([128, 128], bf16, tag="a2T")
                    nc.scalar.activation(a2T[:], h2T[:], Abs)

                    r0 = b * 128
                    xt = spool.tile([128, F], f32, tag="xt")
                    nc.sync.dma_start(xt[:], xshard[r0:r0 + 128, :])
                    h1 = spool.tile([128, F], f32, tag="h1")
                    nc.vector.tensor_scalar_mul(h1[:], xt[:],
                                                deg_all[:, b:b + 1])
                    h1T_ps = ps_tr.tile([128, F], f32, tag="h1T_ps")
                    nc.tensor.transpose(h1T_ps[:], h1[:], ident[:])
                    h1T = spool.tile([128, F], bf16, tag="h1T")
                    nc.vector.tensor_copy(h1T[:], h1T_ps[:])
                    a1T = spool.tile([128, F], bf16, tag="a1T")
                    nc.scalar.activation(a1T[:], h1T_ps[:], Abs)

                    groups = (
                        ("hsum_ps", (("h1T", "wb1"), ("h2T", "wb2"))),
                        ("lpart_ps", (("a1T", "wa1n"), ("a2T", "wa2n"))),
                        ("rpart_ps", (("a1T", "wc1"), ("a2T", "wc2"))),
                    )
                    lhs = {"h1T": h1T, "h2T": h2T, "a1T": a1T, "a2T": a2T}
                    mm = ps_mm.tile([128, 3 * F], f32, tag="mm3")
                    ps_out = {}
                    for kk, (psname, terms) in enumerate(groups):
                        pso = mm[:, kk * F:(kk + 1) * F]
                        for i, (ln, wn) in enumerate(terms):
                            nc.tensor.matmul(
                                pso, lhs[ln][:], wts[wn][:],
                                start=(i == 0), stop=(i == len(terms) - 1))
                        ps_out[psname] = pso
                    ot = opool.tile([128, 3 * F], f32, tag="ot")
                    nc.vector.tensor_add(ot[:, 0:F], bias_bc["bias_c"][:],
                                         ps_out["hsum_ps"])
                    for kk, (pname, bname) in enumerate(
                            (("lpart_ps", "bias_l"), ("rpart_ps", "bias_r"))):
                        tmp = opool.tile([128, F], f32, tag=f"t{kk}")
                        nc.vector.tensor_add(tmp[:], bias_bc[bname][:],
                                             ps_out[pname])
                        nc.vector.tensor_add(
                            ot[:, (kk + 1) * F:(kk + 2) * F],
                            tmp[:], ps_out["hsum_ps"])
                    nc.sync.dma_start(out3[r0:r0 + 128, :], ot[:])

    nc.compile()
    return nc


# ------------------------------------------------------------------- driver
def kernel(X, vertex, edges, X0, n_edges, w_b, w_a, w_c, b_b, b_a, b_c):
    from concourse.bass_utils import run_bass_kernel_spmd
    import ml_dtypes

    X = np.ascontiguousarray(np.asarray(X, dtype=np.float32))
    vertex = np.asarray(vertex).astype(np.int64)
    edges = np.asarray(edges).astype(np.int64)
    w_b = np.asarray(w_b, dtype=np.float32)
    w_a = np.asarray(w_a, dtype=np.float32)
    w_c = np.asarray(w_c, dtype=np.float32)
    b_b = np.asarray(b_b, dtype=np.float32).reshape(1, F)
    b_a = np.asarray(b_a, dtype=np.float32).reshape(1, F)
    b_c = np.asarray(b_c, dtype=np.float32).reshape(1, F)

    r = _route(vertex, edges)
    if r is None:
        return _numpy_fallback(X, vertex, edges, w_b, w_a, w_c, b_b, b_a, b_c)
    (cnt_a, ia_all, ent_a, rowa_all), (cnt_c, ic_all, ent_c, rowc_all) = r

    key = (cnt_a.tobytes(), cnt_c.tobytes(), tuple(ent_a), tuple(ent_c))
    if _STATE.get("key") != key:
        _STATE["nc"] = _build_program(cnt_a, ent_a, len(ent_a),
                                      cnt_c, ent_c, len(ent_c))
        _STATE["key"] = key
    nc = _STATE["nc"]

    Xb = np.ascontiguousarray(X.astype(ml_dtypes.bfloat16))
    deg_full = np.bincount(vertex, minlength=N).astype(np.float32)
    wmats = {
        "wb1": w_b[:F], "wb2": w_b[F:],
        "wa1n": -w_a[:F], "wa2n": -w_a[F:],
        "wc1": w_c[:F], "wc2": w_c[F:],
    }
    bmats = {"bias_c": b_b, "bias_l": b_b - b_a, "bias_r": b_b + b_c}

    in_maps = []
    for m in range(NC):
        xs = np.zeros((NODE_SH_P, F), np.float32)
        xs[:NODE_SH] = X[m * NODE_SH:(m + 1) * NODE_SH]
        dshard = np.zeros(NODE_SH_P, np.float32)
        dshard[:NODE_SH] = deg_full[m * NODE_SH:(m + 1) * NODE_SH]
        im = {
            "xfull": Xb,
            "xshard": xs,
            "ia": ia_all[m], "ic": ic_all[m],
            "rowa": rowa_all[m], "rowc": rowc_all[m],
            "deg": np.ascontiguousarray(dshard.reshape(BLK_C, 128).T),
        }
        for nm, w in wmats.items():
            im[nm] = np.ascontiguousarray(w.astype(ml_dtypes.bfloat16))
        for nm, bv in bmats.items():
            im[nm] = np.ascontiguousarray(bv.astype(np.float32))
        in_maps.append(im)

    res = run_bass_kernel_spmd(nc, in_maps, list(range(NC)))
    full = np.concatenate([res.results[m]["out3"][:NODE_SH]
                           for m in range(NC)])
    full = full.reshape(N, 3, F)
    return (np.ascontiguousarray(full[:, 0]),
            np.ascontiguousarray(full[:, 1]),
            np.ascontiguousarray(full[:, 2]))
